# revision 5
# baseline (speedup 1.0000x reference)
"""Multi-depth attention (BaseMAWAttention) Trainium2 kernel.

Sharding: 8 cores = 4 batches x 2 head-halves (6 heads each). Each core
computes its (batch, head-half) slice end-to-end: QKV projections,
per-(head,depth) scores, softmax, AV. No collectives.

Device layouts (per core):
  hsT   [128, 6, 512]          hidden_states[b].T, k-major (ki, ko, s), bf16
  wq/wk [6, 128, 4, 6, 128]    permuted-transposed Wqd/Wkd: (h, ki, c, ko, f)
                               f' = h*512 + d*64 + e  (head, depth, e ordered)
  wv    [128, 6, 384]          Wv slice transposed (ki, ko, (h e))
  qb/kb [128, 24]              permuted bias slices (per-partition, per chunk)
  out   [6, 2, 128, 4, 4, 64]  (h, dgrp, qp, qchunk, dsub, e), fp32

Pipeline per head: one DMA each for the head's 4 q and 4 k weight
chunks, project chunks into PSUM (6 accumulating matmuls each, DVE
copies to bf16 SBUF), then per depth-group: scoresT[ks,q] =
K'[e,ks].T @ Q'[e,q] into PSUM, the two depths of a pair issued
back-to-back on alternating PE row-groups 0-63/64-127 (same-row-group
back-to-back K=64 matmuls measured 1.7x slower); exp via ScalarE on
[128,1024] PSUM tiles (scale=1/8, per-partition mask bias) -> bf16
SBUF; AV with V augmented by a ones column so the same matmul yields
the softmax denominator in column 64; DVE reciprocal + broadcast-
multiply normalizes into query-chunk-pair tiles; one DMA out per pair.

All matmuls bf16 (fp32 runs at 1/4 rate on the PE; float32r and fp8
rejected: fp8 fails the 2e-2 accuracy budget, f32r measured slower).
Softmax needs no max-subtraction: scores are O(1) and the mask enters
as an additive bias, which cancels in the softmax ratio exactly like
the reference's where(mask==0, -1e9) + max-subtraction.
"""

import os
import sys

import numpy as np

try:
    import concourse.bass as bass  # noqa: F401
except ImportError:
    sys.path.insert(0, "/opt/trn_rl_repo")

import ml_dtypes

HIDDEN = 768
HEADS = 12
HD = 64
DEPTH = 8
B = 4
S = 512
N_CORES = 8
HB = HEADS // 2          # heads per core
P = 128
NKC = HIDDEN // P        # 6 contraction chunks
FC = HB * DEPTH * HD // P  # 24 feature chunks of Q'/K'
KSC = S // P             # 4 key/seq chunks
F = FC * P               # 3072

_BF = ml_dtypes.bfloat16

_cache = {}

DEFAULT_OPTS = dict(
    qk_bufs=2,        # SBUF bufs per q/k tag
    sc_bufs=2,        # PSUM score-tile bufs (2 banks each)
    av_in_pp=False,   # AV accumulates in the proj PSUM pool (measured bad)
    dgrp_loop=False,  # (head, depth-group)-granular proj->attention pipeline
    interleave=False,  # q,k,q,k projection order within a group
    copy_eng="vector",  # engine for Q/K PSUM->SBUF copies (GPSIMD can't read PSUM)
    ep_bufs=6,
    wts_bufs=2,       # per-head weight tile bufs (per q/k tag)
    ob_bufs=3,
    pp_bufs=2,
    batch_out=True,   # one output DMA per query-chunk pair
    consts_bufs=2,    # double-buffer hsT/wv/V across loop iterations
)


def _build(use_mask, use_qk_bias, reps=1, probe=None, tune=0, opts=None):
    import contextlib

    import concourse.bacc as bacc
    import concourse.mybir as mybir
    import concourse.tile as tile

    o = dict(DEFAULT_OPTS)
    if opts:
        o.update(opts)

    f32 = mybir.dt.float32
    bf = mybir.dt.bfloat16
    Exp = mybir.ActivationFunctionType.Exp

    nc = bacc.Bacc(
        "TRN2", target_bir_lowering=False, debug=False, num_devices=N_CORES
    )
    hsT_d = nc.dram_tensor("hsT", [P, NKC, S], bf, kind="ExternalInput")
    wq_d = nc.dram_tensor("wq", [HB, P, 4, NKC, P], bf, kind="ExternalInput")
    wk_d = nc.dram_tensor("wk", [HB, P, 4, NKC, P], bf, kind="ExternalInput")
    wv_d = nc.dram_tensor("wv", [P, NKC, HB * HD], bf, kind="ExternalInput")
    if use_qk_bias:
        qb_d = nc.dram_tensor("qb", [P, FC], f32, kind="ExternalInput")
        kb_d = nc.dram_tensor("kb", [P, FC], f32, kind="ExternalInput")
    if use_mask:
        mb_d = nc.dram_tensor("mb", [P, KSC], f32, kind="ExternalInput")
    out_d = nc.dram_tensor(
        "out", [HB, 2, P, KSC, 4, HD], f32, kind="ExternalOutput"
    )

    with tile.TileContext(nc) as tc:
        with (
            tc.tile_pool(name="consts", bufs=o["consts_bufs"]) as consts,
            tc.tile_pool(name="wts", bufs=o["wts_bufs"]) as wts,
            tc.tile_pool(name="qk", bufs=o["qk_bufs"]) as qk,
            tc.tile_pool(name="ep", bufs=o["ep_bufs"]) as ep,
            tc.tile_pool(name="ob", bufs=o["ob_bufs"]) as ob,
            tc.tile_pool(name="pp", bufs=o["pp_bufs"], space="PSUM") as ps_pp,
            tc.tile_pool(name="sc", bufs=o["sc_bufs"], space="PSUM") as ps_sc,
            (
                tc.tile_pool(name="av", bufs=2, space="PSUM")
                if not o["av_in_pp"]
                else contextlib.nullcontext()
            ) as _ps_av,
            tc.For_i(0, reps, 1) if reps > 1 else contextlib.nullcontext(),
        ):
            copy_eng = getattr(nc, o["copy_eng"])
            hsT_sb = consts.tile([P, NKC, S], bf, tag="hsT")
            nc.sync.dma_start(hsT_sb[:], hsT_d.ap())
            wv_sb = consts.tile([P, NKC, HB * HD], bf, tag="wv")
            nc.sync.dma_start(wv_sb[:], wv_d.ap())
            if use_qk_bias:
                qb_sb = consts.tile([P, FC], f32, tag="qb")
                nc.sync.dma_start(qb_sb[:], qb_d.ap())
                kb_sb = consts.tile([P, FC], f32, tag="kb")
                nc.sync.dma_start(kb_sb[:], kb_d.ap())
            if use_mask:
                mb_sb = consts.tile([P, KSC], f32, tag="mb")
                nc.sync.dma_start(mb_sb[:], mb_d.ap())

            v_sb = consts.tile([P, KSC, HB, HD + 1], bf, tag="v")
            nc.vector.memset(v_sb[:, :, :, HD : HD + 1], 1.0)

            # V projection first: psum[s128, (h e)384] = hsT_chunk.T @ wv
            for sc_ in range(KSC):
                ps = ps_pp.tile([P, S], f32, tag="pp")
                for ko in range(NKC):
                    nc.tensor.matmul(
                        ps[:, : HB * HD],
                        hsT_sb[:, ko, sc_ * P : (sc_ + 1) * P],
                        wv_sb[:, ko, :],
                        start=(ko == 0),
                        stop=(ko == NKC - 1),
                    )
                nc.vector.tensor_copy(
                    v_sb[:, sc_, :, 0:HD],
                    ps[:, : HB * HD].rearrange("p (h e) -> p h e", e=HD),
                )

            def project(wt, dst, b_name, h, lc, li):
                # wt[:, lc] holds chunk c = h*4+lc; write dst[:, li, :]
                ps = ps_pp.tile([P, S], f32, tag="pp")
                n_proj = 2 if probe == "proj2" else 1
                if probe == "projnone":
                    n_proj = 0
                    nc.vector.memset(ps[:, 0:4], 0.5)
                for _rep in range(n_proj):
                    for ko in range(NKC):
                        nc.tensor.matmul(
                            ps[:],
                            wt[:, lc, ko, :],
                            hsT_sb[:, ko, :],
                            start=(ko == 0),
                            stop=(ko == NKC - 1),
                        )
                if use_qk_bias:
                    c = h * 4 + lc
                    b_sb = qb_sb if b_name == "q" else kb_sb
                    nc.vector.tensor_scalar_add(
                        dst[:, li, :], ps[:], b_sb[:, c : c + 1]
                    )
                else:
                    copy_eng.tensor_copy(dst[:, li, :], ps[:])

            def attention(h, dgrp, q_t, k_t, li_of):
                # li_of[pi] = index into q_t/k_t's chunk dim for depth-pair pi
                e_pairs = []
                for pi in range(2):
                    li = li_of[pi]
                    e_p = ep.tile([P, KSC, 2, S], bf, tag="exp")
                    for ksc in range(KSC):
                        sps = ps_sc.tile([P, 2, S], f32, tag="sc")
                        n_sc = 2 if probe == "sc2" else 1
                        if probe == "scnone":
                            n_sc = 0
                            nc.vector.memset(sps[:, :, 0:4], 0.5)
                        for _rep in range(n_sc):
                            for pd in range(2):
                                base = pd * 64
                                nc.tensor.matmul(
                                    sps[:, pd, :],
                                    k_t[
                                        base : base + 64,
                                        li,
                                        ksc * P : (ksc + 1) * P,
                                    ],
                                    q_t[base : base + 64, li, :],
                                    start=True,
                                    stop=True,
                                )
                        if probe == "exp2":
                            e_x = ep.tile([P, 2, S], bf, tag="exp2")
                            nc.scalar.activation(
                                e_x[:], sps[:], Exp, scale=0.125
                            )
                        if probe == "exphalf":
                            if ksc == 0:
                                nc.vector.memset(e_p[:, :, 1, 0:4], 0.5)
                            nc.scalar.activation(
                                e_p[:, ksc, 0, :],
                                sps[:, 0, :],
                                Exp,
                                scale=0.125,
                            )
                        elif use_mask:
                            nc.scalar.activation(
                                e_p[:, ksc, :, :],
                                sps[:],
                                Exp,
                                bias=mb_sb[:, ksc : ksc + 1],
                                scale=0.125,
                            )
                        else:
                            nc.scalar.activation(
                                e_p[:, ksc, :, :],
                                sps[:],
                                Exp,
                                scale=0.125,
                            )
                    e_pairs.append(e_p)
                if probe == "noav":
                    return
                for qcp in range(2) if o["batch_out"] else range(4):
                    if o["batch_out"]:
                        o_t = ob.tile([P, 2, 4, HD], f32, tag="o")
                        qcs = (qcp * 2, qcp * 2 + 1)
                    else:
                        o_t = ob.tile([P, 1, 4, HD], f32, tag="o")
                        qcs = (qcp,)
                    for oi, qc in enumerate(qcs):
                        if o["av_in_pp"]:
                            avf = ps_pp.tile([P, S], f32, tag="pp")
                            av = avf[:, : 4 * (HD + 1)].rearrange(
                                "p (i e) -> p i e", e=HD + 1
                            )
                        else:
                            av = _ps_av.tile([P, 4, HD + 1], f32, tag="av")
                        for i in range(4):
                            pi, pd = i // 2, i % 2
                            for ksc in range(KSC):
                                nc.tensor.matmul(
                                    av[:, i, :],
                                    e_pairs[pi][
                                        :, ksc, pd, qc * P : (qc + 1) * P
                                    ],
                                    v_sb[:, ksc, h, :],
                                    start=(ksc == 0),
                                    stop=(ksc == KSC - 1),
                                )
                        r = ob.tile([P, 4], f32, tag="r")
                        nc.vector.reciprocal(r[:], av[:, :, HD])
                        nc.vector.tensor_mul(
                            o_t[:, oi],
                            av[:, :, 0:HD],
                            r[:, :, None].to_broadcast([P, 4, HD]),
                        )
                    nc.sync.dma_start(
                        out_d.ap()[h, dgrp][
                            :, qcs[0] : qcs[-1] + 1
                        ],
                        o_t[:, : len(qcs)],
                    )

            if o["dgrp_loop"]:
                for h in range(HB):
                    for dgrp in range(2):
                        wt_q = wts.tile([P, 2, NKC, P], bf, tag="wq")
                        nc.sync.dma_start(
                            wt_q[:], wq_d.ap()[h][:, dgrp * 2 : dgrp * 2 + 2]
                        )
                        wt_k = wts.tile([P, 2, NKC, P], bf, tag="wk")
                        nc.sync.dma_start(
                            wt_k[:], wk_d.ap()[h][:, dgrp * 2 : dgrp * 2 + 2]
                        )
                        q_t = qk.tile([P, 2, S], bf, tag="q")
                        k_t = qk.tile([P, 2, S], bf, tag="k")
                        for pi in range(2):
                            lc = dgrp * 2 + pi
                            project(wt_q, q_t, "q", h, pi, pi)
                            project(wt_k, k_t, "k", h, pi, pi)
                        attention(h, dgrp, q_t, k_t, li_of=[0, 1])
            else:
                for h in range(HB):
                    wt_q = wts.tile([P, 4, NKC, P], bf, tag="wq")
                    nc.sync.dma_start(wt_q[:], wq_d.ap()[h])
                    wt_k = wts.tile([P, 4, NKC, P], bf, tag="wk")
                    nc.sync.dma_start(wt_k[:], wk_d.ap()[h])
                    q_t = qk.tile([P, 4, S], bf, tag="q")
                    k_t = qk.tile([P, 4, S], bf, tag="k")
                    if o["interleave"]:
                        for lc in range(4):
                            project(wt_q, q_t, "q", h, lc, lc)
                            project(wt_k, k_t, "k", h, lc, lc)
                    else:
                        for lc in range(4):
                            project(wt_q, q_t, "q", h, lc, lc)
                        for lc in range(4):
                            project(wt_k, k_t, "k", h, lc, lc)
                    for dgrp in range(2):
                        attention(
                            h, dgrp, q_t, k_t, li_of=[dgrp * 2, dgrp * 2 + 1]
                        )

    nc.compile()
    return nc


def _get_program(use_mask, use_qk_bias):
    key = (use_mask, use_qk_bias)
    if key not in _cache:
        _cache[key] = _build(use_mask, use_qk_bias)
    return _cache[key]


def _perm_idx(h0):
    # f' = h*512 + d*64 + e maps to original row ((h0+h)*64+e)*8 + d
    idx = np.empty(F, dtype=np.int64)
    f = 0
    for h in range(HB):
        for d in range(DEPTH):
            for e in range(HD):
                idx[f] = ((h0 + h) * HD + e) * DEPTH + d
                f += 1
    return idx


def _prep_w(Wd, idx):
    # [6144,768] -> permuted rows [3072,768] -> (h, ki, c, ko, f)
    A = np.ascontiguousarray(Wd[idx])  # [3072, 768]
    chunks = A.reshape(FC, P, NKC, P).transpose(0, 3, 2, 1)  # (fc, ki, ko, f)
    return np.ascontiguousarray(
        chunks.reshape(HB, 4, P, NKC, P).transpose(0, 2, 1, 3, 4)
    ).astype(_BF)


def _prep_hsT(hs_b):
    # [512, 768] -> [768,512] -> (ki, ko, s)
    return np.ascontiguousarray(
        hs_b.T.reshape(NKC, P, S).transpose(1, 0, 2)
    ).astype(_BF)


def _prep_wv(Wv, h0):
    Wvs = Wv[h0 * HD : (h0 + HB) * HD]  # [384, 768]
    return np.ascontiguousarray(
        Wvs.T.reshape(NKC, P, HB * HD).transpose(1, 0, 2)
    ).astype(_BF)


last_results = None


def kernel(
    hidden_states,
    attention_mask,
    Wq,
    bq,
    Wk,
    bk,
    Wv,
    bv,
    Wqd,
    bqd,
    Wkd,
    bkd,
):
    global last_results
    from concourse.bass_utils import run_bass_kernel_spmd

    hs = np.asarray(hidden_states, dtype=np.float32)
    mask = np.asarray(attention_mask)
    Wv = np.asarray(Wv, dtype=np.float32)
    bv = np.asarray(bv, dtype=np.float32)
    Wqd = np.asarray(Wqd, dtype=np.float32)
    bqd = np.asarray(bqd, dtype=np.float32)
    Wkd = np.asarray(Wkd, dtype=np.float32)
    bkd = np.asarray(bkd, dtype=np.float32)

    use_mask = not bool(np.all(mask != 0))
    use_qk_bias = bool(np.any(bqd) or np.any(bkd))
    nc = _get_program(use_mask, use_qk_bias)

    idx = [_perm_idx(0), _perm_idx(HB)]
    wq_p = [_prep_w(Wqd, idx[hh]) for hh in range(2)]
    wk_p = [_prep_w(Wkd, idx[hh]) for hh in range(2)]
    wv_p = [_prep_wv(Wv, hh * HB) for hh in range(2)]
    qb_p = [
        np.ascontiguousarray(bqd[idx[hh]].reshape(FC, P).T).astype(np.float32)
        for hh in range(2)
    ]
    kb_p = [
        np.ascontiguousarray(bkd[idx[hh]].reshape(FC, P).T).astype(np.float32)
        for hh in range(2)
    ]

    in_maps = []
    for c in range(N_CORES):
        b, hh = c // 2, c % 2
        m = {
            "hsT": _prep_hsT(hs[b]),
            "wq": wq_p[hh],
            "wk": wk_p[hh],
            "wv": wv_p[hh],
        }
        if use_qk_bias:
            m["qb"] = qb_p[hh]
            m["kb"] = kb_p[hh]
        if use_mask:
            mb = np.where(mask[b] == 0, np.float32(-1e9), np.float32(0.0))
            m["mb"] = np.ascontiguousarray(
                mb.reshape(KSC, P).T
            ).astype(np.float32)
        in_maps.append(m)

    res = run_bass_kernel_spmd(nc, in_maps, list(range(N_CORES)))
    last_results = res

    out = np.empty((DEPTH, B, HEADS, S, HD), dtype=np.float32)
    for c in range(N_CORES):
        b, hh = c // 2, c % 2
        arr = res.results[c]["out"]  # [6, 2, 128, 4, 4, 64] (h,dgrp,qp,qc,i,e)
        a = np.ascontiguousarray(arr.transpose(1, 4, 0, 3, 2, 5)).reshape(
            DEPTH, HB, S, HD
        )
        out[:, b, hh * HB : (hh + 1) * HB] = a
    if np.any(bv):
        out += bv.reshape(HEADS, HD)[None, None, :, None, :]
    return out


# revision 8
# speedup vs baseline: 1.0133x; 1.0133x over previous
"""Multi-depth attention (BaseMAWAttention) Trainium2 kernel.

Sharding: 8 cores = 4 batches x 2 head-halves (6 heads each). Each core
computes its (batch, head-half) slice end-to-end: QKV projections,
per-(head,depth) scores, softmax, AV. No collectives.

Device layouts (per core):
  hsT   [128, 6, 512]          hidden_states[b].T, k-major (ki, ko, s), bf16
  wq/wk [6, 128, 4, 6, 128]    permuted-transposed Wqd/Wkd: (h, ki, c, ko, f)
                               f' = h*512 + d*64 + e  (head, depth, e ordered)
  wv    [128, 6, 384]          Wv slice transposed (ki, ko, (h e))
  qb/kb [128, 24]              permuted bias slices (per-partition, per chunk)
  out   [6, 2, 128, 4, 4, 64]  (h, dgrp, qp, qchunk, dsub, e), fp32

Pipeline per head: one DMA each for the head's 4 q and 4 k weight
chunks, project chunks into PSUM (6 accumulating matmuls each, DVE
copies to bf16 SBUF), then per depth-group: scoresT[ks,q] =
K'[e,ks].T @ Q'[e,q] into PSUM, the two depths of a pair issued
back-to-back on alternating PE row-groups 0-63/64-127 (same-row-group
back-to-back K=64 matmuls measured 1.7x slower); exp via ScalarE on
[128,1024] PSUM tiles (scale=1/8, per-partition mask bias) -> bf16
SBUF; AV with V augmented by a ones column so the same matmul yields
the softmax denominator in column 64; DVE reciprocal + broadcast-
multiply normalizes into query-chunk-pair tiles; one DMA out per pair.

All matmuls bf16 (fp32 runs at 1/4 rate on the PE; float32r and fp8
rejected: fp8 fails the 2e-2 accuracy budget, f32r measured slower).
Softmax needs no max-subtraction: scores are O(1) and the mask enters
as an additive bias, which cancels in the softmax ratio exactly like
the reference's where(mask==0, -1e9) + max-subtraction.
"""

import os
import sys

import numpy as np

try:
    import concourse.bass as bass  # noqa: F401
except ImportError:
    sys.path.insert(0, "/opt/trn_rl_repo")

import ml_dtypes

HIDDEN = 768
HEADS = 12
HD = 64
DEPTH = 8
B = 4
S = 512
N_CORES = 8
HB = HEADS // 2          # heads per core
P = 128
NKC = HIDDEN // P        # 6 contraction chunks
FC = HB * DEPTH * HD // P  # 24 feature chunks of Q'/K'
KSC = S // P             # 4 key/seq chunks
F = FC * P               # 3072

_BF = ml_dtypes.bfloat16

_cache = {}

DEFAULT_OPTS = dict(
    qk_bufs=2,        # SBUF bufs per q/k tag
    sc_bufs=2,        # PSUM score-tile bufs (2 banks each)
    av_in_pp=False,   # AV accumulates in the proj PSUM pool (measured bad)
    dgrp_loop=False,  # (head, depth-group)-granular proj->attention pipeline
    interleave=False,  # q,k,q,k projection order within a group
    copy_eng="vector",  # engine for Q/K PSUM->SBUF copies (GPSIMD can't read PSUM)
    ep_bufs=6,
    wts_bufs=2,       # per-head weight tile bufs (per q/k tag)
    ob_bufs=3,
    pp_bufs=2,
    batch_out=True,   # one output DMA per query-chunk pair
    consts_bufs=2,    # double-buffer hsT/wv/V across loop iterations
)


def _build(use_mask, use_qk_bias, reps=1, probe=None, tune=0, opts=None):
    import contextlib

    import concourse.bacc as bacc
    import concourse.mybir as mybir
    import concourse.tile as tile

    o = dict(DEFAULT_OPTS)
    if opts:
        o.update(opts)

    f32 = mybir.dt.float32
    bf = mybir.dt.bfloat16
    Exp = mybir.ActivationFunctionType.Exp

    nc = bacc.Bacc(
        "TRN2", target_bir_lowering=False, debug=False, num_devices=N_CORES
    )
    hsT_d = nc.dram_tensor("hsT", [P, NKC, S], bf, kind="ExternalInput")
    wq_d = nc.dram_tensor("wq", [HB, P, 4, NKC, P], bf, kind="ExternalInput")
    wk_d = nc.dram_tensor("wk", [HB, P, 4, NKC, P], bf, kind="ExternalInput")
    wv_d = nc.dram_tensor("wv", [P, NKC, HB * HD], bf, kind="ExternalInput")
    if use_qk_bias:
        qb_d = nc.dram_tensor("qb", [P, FC], f32, kind="ExternalInput")
        kb_d = nc.dram_tensor("kb", [P, FC], f32, kind="ExternalInput")
    if use_mask:
        mb_d = nc.dram_tensor("mb", [P, KSC], f32, kind="ExternalInput")
    out_d = nc.dram_tensor(
        "out", [HB, 2, P, KSC, 4, HD], f32, kind="ExternalOutput"
    )

    with tile.TileContext(nc) as tc:
        with (
            tc.tile_pool(name="consts", bufs=o["consts_bufs"]) as consts,
            tc.tile_pool(name="wts", bufs=o["wts_bufs"]) as wts,
            tc.tile_pool(name="qk", bufs=o["qk_bufs"]) as qk,
            tc.tile_pool(name="ep", bufs=o["ep_bufs"]) as ep,
            tc.tile_pool(name="ob", bufs=o["ob_bufs"]) as ob,
            tc.tile_pool(name="pp", bufs=o["pp_bufs"], space="PSUM") as ps_pp,
            tc.tile_pool(name="sc", bufs=o["sc_bufs"], space="PSUM") as ps_sc,
            (
                tc.tile_pool(name="av", bufs=2, space="PSUM")
                if not o["av_in_pp"]
                else contextlib.nullcontext()
            ) as _ps_av,
            tc.For_i(0, reps, 1) if reps > 1 else contextlib.nullcontext(),
        ):
            copy_eng = getattr(nc, o["copy_eng"])
            hsT_sb = consts.tile([P, NKC, S], bf, tag="hsT")
            nc.sync.dma_start(hsT_sb[:], hsT_d.ap())
            wv_sb = consts.tile([P, NKC, HB * HD], bf, tag="wv")
            nc.sync.dma_start(wv_sb[:], wv_d.ap())
            if use_qk_bias:
                qb_sb = consts.tile([P, FC], f32, tag="qb")
                nc.sync.dma_start(qb_sb[:], qb_d.ap())
                kb_sb = consts.tile([P, FC], f32, tag="kb")
                nc.sync.dma_start(kb_sb[:], kb_d.ap())
            if use_mask:
                mb_sb = consts.tile([P, KSC], f32, tag="mb")
                nc.sync.dma_start(mb_sb[:], mb_d.ap())

            v_sb = consts.tile([P, KSC, HB, HD + 1], bf, tag="v")
            nc.vector.memset(v_sb[:, :, :, HD : HD + 1], 1.0)

            # V projection first: psum[s128, (h e)384] = hsT_chunk.T @ wv
            for sc_ in range(KSC):
                ps = ps_pp.tile([P, S], f32, tag="pp")
                for ko in range(NKC):
                    nc.tensor.matmul(
                        ps[:, : HB * HD],
                        hsT_sb[:, ko, sc_ * P : (sc_ + 1) * P],
                        wv_sb[:, ko, :],
                        start=(ko == 0),
                        stop=(ko == NKC - 1),
                    )
                nc.vector.tensor_copy(
                    v_sb[:, sc_, :, 0:HD],
                    ps[:, : HB * HD].rearrange("p (h e) -> p h e", e=HD),
                )

            def project(wt, dst, b_name, h, lc, li):
                # wt[:, lc] holds chunk c = h*4+lc; write dst[:, li, :]
                ps = ps_pp.tile([P, S], f32, tag="pp")
                n_proj = 2 if probe == "proj2" else 1
                if probe == "projnone":
                    n_proj = 0
                    nc.vector.memset(ps[:, 0:4], 0.5)
                for _rep in range(n_proj):
                    for ko in range(NKC):
                        nc.tensor.matmul(
                            ps[:],
                            wt[:, lc, ko, :],
                            hsT_sb[:, ko, :],
                            start=(ko == 0),
                            stop=(ko == NKC - 1),
                        )
                if use_qk_bias:
                    c = h * 4 + lc
                    b_sb = qb_sb if b_name == "q" else kb_sb
                    nc.vector.tensor_scalar_add(
                        dst[:, li, :], ps[:], b_sb[:, c : c + 1]
                    )
                else:
                    copy_eng.tensor_copy(dst[:, li, :], ps[:])

            def scores_phase(h, dgrp, q_t, k_t, li_of):
                # li_of[pi] = index into q_t/k_t's chunk dim for depth-pair pi
                e_pairs = []
                for pi in range(2):
                    li = li_of[pi]
                    e_p = ep.tile([P, KSC, 2, S], bf, tag="exp")
                    for ksc in range(KSC):
                        sps = ps_sc.tile([P, 2, S], f32, tag="sc")
                        n_sc = 2 if probe == "sc2" else 1
                        if probe == "scnone":
                            n_sc = 0
                            nc.vector.memset(sps[:, :, 0:4], 0.5)
                        for _rep in range(n_sc):
                            for pd in range(2):
                                base = pd * 64
                                nc.tensor.matmul(
                                    sps[:, pd, :],
                                    k_t[
                                        base : base + 64,
                                        li,
                                        ksc * P : (ksc + 1) * P,
                                    ],
                                    q_t[base : base + 64, li, :],
                                    start=True,
                                    stop=True,
                                )
                        if probe == "exp2":
                            e_x = ep.tile([P, 2, S], bf, tag="exp2")
                            nc.scalar.activation(
                                e_x[:], sps[:], Exp, scale=0.125
                            )
                        if probe == "exphalf":
                            if ksc == 0:
                                nc.vector.memset(e_p[:, :, 1, 0:4], 0.5)
                            nc.scalar.activation(
                                e_p[:, ksc, 0, :],
                                sps[:, 0, :],
                                Exp,
                                scale=0.125,
                            )
                        elif use_mask:
                            nc.scalar.activation(
                                e_p[:, ksc, :, :],
                                sps[:],
                                Exp,
                                bias=mb_sb[:, ksc : ksc + 1],
                                scale=0.125,
                            )
                        else:
                            nc.scalar.activation(
                                e_p[:, ksc, :, :],
                                sps[:],
                                Exp,
                                scale=0.125,
                            )
                    e_pairs.append(e_p)
                return e_pairs

            def av_phase(h, dgrp, e_pairs):
                for qcp in range(2) if o["batch_out"] else range(4):
                    if o["batch_out"]:
                        o_t = ob.tile([P, 2, 4, HD], f32, tag="o")
                        qcs = (qcp * 2, qcp * 2 + 1)
                    else:
                        o_t = ob.tile([P, 1, 4, HD], f32, tag="o")
                        qcs = (qcp,)
                    for oi, qc in enumerate(qcs):
                        if o["av_in_pp"]:
                            avf = ps_pp.tile([P, S], f32, tag="pp")
                            av = avf[:, : 4 * (HD + 1)].rearrange(
                                "p (i e) -> p i e", e=HD + 1
                            )
                        else:
                            av = _ps_av.tile([P, 4, HD + 1], f32, tag="av")
                        for i in range(4):
                            pi, pd = i // 2, i % 2
                            for ksc in range(KSC):
                                nc.tensor.matmul(
                                    av[:, i, :],
                                    e_pairs[pi][
                                        :, ksc, pd, qc * P : (qc + 1) * P
                                    ],
                                    v_sb[:, ksc, h, :],
                                    start=(ksc == 0),
                                    stop=(ksc == KSC - 1),
                                )
                        r = ob.tile([P, 4], f32, tag="r")
                        nc.vector.reciprocal(r[:], av[:, :, HD])
                        nc.vector.tensor_mul(
                            o_t[:, oi],
                            av[:, :, 0:HD],
                            r[:, :, None].to_broadcast([P, 4, HD]),
                        )
                    nc.sync.dma_start(
                        out_d.ap()[h, dgrp][
                            :, qcs[0] : qcs[-1] + 1
                        ],
                        o_t[:, : len(qcs)],
                    )

            def attention(h, dgrp, q_t, k_t, li_of):
                e_pairs = scores_phase(h, dgrp, q_t, k_t, li_of)
                if probe == "noav":
                    return
                av_phase(h, dgrp, e_pairs)

            if o.get("sw_pipe"):
                # Software-pipelined issue order: projection units are slotted
                # between scores(d) and AV(d) so the in-order PE queue has
                # filler work while the ScalarE exp tail completes; otherwise
                # AV blocks the queue head for the exp latency every dgrp.
                def alloc_head(h):
                    wt_q = wts.tile([P, 4, NKC, P], bf, tag="wq")
                    nc.sync.dma_start(wt_q[:], wq_d.ap()[h])
                    wt_k = wts.tile([P, 4, NKC, P], bf, tag="wk")
                    nc.sync.dma_start(wt_k[:], wk_d.ap()[h])
                    q_t = qk.tile([P, 4, S], bf, tag="q")
                    k_t = qk.tile([P, 4, S], bf, tag="k")
                    return wt_q, wt_k, q_t, k_t

                def proj_pair(h, lc, tl):
                    project(tl[0], tl[2], "q", h, lc, lc)
                    project(tl[1], tl[3], "k", h, lc, lc)

                tl = alloc_head(0)
                proj_pair(0, 0, tl)
                proj_pair(0, 1, tl)
                ntl = None
                for h in range(HB):
                    for d in range(2):
                        e = scores_phase(
                            h, d, tl[2], tl[3], li_of=[2 * d, 2 * d + 1]
                        )
                        if d == 0:
                            proj_pair(h, 2, tl)
                            proj_pair(h, 3, tl)
                        elif h + 1 < HB:
                            ntl = alloc_head(h + 1)
                            proj_pair(h + 1, 0, ntl)
                            proj_pair(h + 1, 1, ntl)
                        if probe != "noav":
                            av_phase(h, d, e)
                    if h + 1 < HB:
                        tl = ntl
            elif o["dgrp_loop"]:
                for h in range(HB):
                    for dgrp in range(2):
                        wt_q = wts.tile([P, 2, NKC, P], bf, tag="wq")
                        nc.sync.dma_start(
                            wt_q[:], wq_d.ap()[h][:, dgrp * 2 : dgrp * 2 + 2]
                        )
                        wt_k = wts.tile([P, 2, NKC, P], bf, tag="wk")
                        nc.sync.dma_start(
                            wt_k[:], wk_d.ap()[h][:, dgrp * 2 : dgrp * 2 + 2]
                        )
                        q_t = qk.tile([P, 2, S], bf, tag="q")
                        k_t = qk.tile([P, 2, S], bf, tag="k")
                        for pi in range(2):
                            lc = dgrp * 2 + pi
                            project(wt_q, q_t, "q", h, pi, pi)
                            project(wt_k, k_t, "k", h, pi, pi)
                        attention(h, dgrp, q_t, k_t, li_of=[0, 1])
            else:
                for h in range(HB):
                    wt_q = wts.tile([P, 4, NKC, P], bf, tag="wq")
                    nc.sync.dma_start(wt_q[:], wq_d.ap()[h])
                    wt_k = wts.tile([P, 4, NKC, P], bf, tag="wk")
                    nc.sync.dma_start(wt_k[:], wk_d.ap()[h])
                    q_t = qk.tile([P, 4, S], bf, tag="q")
                    k_t = qk.tile([P, 4, S], bf, tag="k")
                    if o["interleave"]:
                        for lc in range(4):
                            project(wt_q, q_t, "q", h, lc, lc)
                            project(wt_k, k_t, "k", h, lc, lc)
                    else:
                        for lc in range(4):
                            project(wt_q, q_t, "q", h, lc, lc)
                        for lc in range(4):
                            project(wt_k, k_t, "k", h, lc, lc)
                    for dgrp in range(2):
                        attention(
                            h, dgrp, q_t, k_t, li_of=[dgrp * 2, dgrp * 2 + 1]
                        )

    nc.compile()
    return nc


def _get_program(use_mask, use_qk_bias):
    key = (use_mask, use_qk_bias)
    if key not in _cache:
        _cache[key] = _build(use_mask, use_qk_bias)
    return _cache[key]


def _perm_idx(h0):
    # f' = h*512 + d*64 + e maps to original row ((h0+h)*64+e)*8 + d
    idx = np.empty(F, dtype=np.int64)
    f = 0
    for h in range(HB):
        for d in range(DEPTH):
            for e in range(HD):
                idx[f] = ((h0 + h) * HD + e) * DEPTH + d
                f += 1
    return idx


def _prep_w(Wd, idx):
    # [6144,768] -> permuted rows [3072,768] -> (h, ki, c, ko, f)
    A = np.ascontiguousarray(Wd[idx])  # [3072, 768]
    chunks = A.reshape(FC, P, NKC, P).transpose(0, 3, 2, 1)  # (fc, ki, ko, f)
    return np.ascontiguousarray(
        chunks.reshape(HB, 4, P, NKC, P).transpose(0, 2, 1, 3, 4)
    ).astype(_BF)


def _prep_hsT(hs_b):
    # [512, 768] -> [768,512] -> (ki, ko, s)
    return np.ascontiguousarray(
        hs_b.T.reshape(NKC, P, S).transpose(1, 0, 2)
    ).astype(_BF)


def _prep_wv(Wv, h0):
    Wvs = Wv[h0 * HD : (h0 + HB) * HD]  # [384, 768]
    return np.ascontiguousarray(
        Wvs.T.reshape(NKC, P, HB * HD).transpose(1, 0, 2)
    ).astype(_BF)


last_results = None


def kernel(
    hidden_states,
    attention_mask,
    Wq,
    bq,
    Wk,
    bk,
    Wv,
    bv,
    Wqd,
    bqd,
    Wkd,
    bkd,
):
    global last_results
    from concourse.bass_utils import run_bass_kernel_spmd

    hs = np.asarray(hidden_states, dtype=np.float32)
    mask = np.asarray(attention_mask)
    Wv = np.asarray(Wv, dtype=np.float32)
    bv = np.asarray(bv, dtype=np.float32)
    Wqd = np.asarray(Wqd, dtype=np.float32)
    bqd = np.asarray(bqd, dtype=np.float32)
    Wkd = np.asarray(Wkd, dtype=np.float32)
    bkd = np.asarray(bkd, dtype=np.float32)

    use_mask = not bool(np.all(mask != 0))
    use_qk_bias = bool(np.any(bqd) or np.any(bkd))
    nc = _get_program(use_mask, use_qk_bias)

    idx = [_perm_idx(0), _perm_idx(HB)]
    wq_p = [_prep_w(Wqd, idx[hh]) for hh in range(2)]
    wk_p = [_prep_w(Wkd, idx[hh]) for hh in range(2)]
    wv_p = [_prep_wv(Wv, hh * HB) for hh in range(2)]
    qb_p = [
        np.ascontiguousarray(bqd[idx[hh]].reshape(FC, P).T).astype(np.float32)
        for hh in range(2)
    ]
    kb_p = [
        np.ascontiguousarray(bkd[idx[hh]].reshape(FC, P).T).astype(np.float32)
        for hh in range(2)
    ]

    in_maps = []
    for c in range(N_CORES):
        b, hh = c // 2, c % 2
        m = {
            "hsT": _prep_hsT(hs[b]),
            "wq": wq_p[hh],
            "wk": wk_p[hh],
            "wv": wv_p[hh],
        }
        if use_qk_bias:
            m["qb"] = qb_p[hh]
            m["kb"] = kb_p[hh]
        if use_mask:
            mb = np.where(mask[b] == 0, np.float32(-1e9), np.float32(0.0))
            m["mb"] = np.ascontiguousarray(
                mb.reshape(KSC, P).T
            ).astype(np.float32)
        in_maps.append(m)

    res = run_bass_kernel_spmd(nc, in_maps, list(range(N_CORES)))
    last_results = res

    out = np.empty((DEPTH, B, HEADS, S, HD), dtype=np.float32)
    for c in range(N_CORES):
        b, hh = c // 2, c % 2
        arr = res.results[c]["out"]  # [6, 2, 128, 4, 4, 64] (h,dgrp,qp,qc,i,e)
        a = np.ascontiguousarray(arr.transpose(1, 4, 0, 3, 2, 5)).reshape(
            DEPTH, HB, S, HD
        )
        out[:, b, hh * HB : (hh + 1) * HB] = a
    if np.any(bv):
        out += bv.reshape(HEADS, HD)[None, None, :, None, :]
    return out


# revision 9
# speedup vs baseline: 1.0241x; 1.0106x over previous
"""Multi-depth attention (BaseMAWAttention) Trainium2 kernel.

Sharding: 8 cores = 4 batches x 2 head-halves (6 heads each). Each core
computes its (batch, head-half) slice end-to-end: QKV projections,
per-(head,depth) scores, softmax, AV. No collectives.

Device layouts (per core):
  hsT   [128, 6, 512]          hidden_states[b].T, k-major (ki, ko, s), bf16
  wq/wk [6, 128, 4, 6, 128]    permuted-transposed Wqd/Wkd: (h, ki, c, ko, f)
                               f' = h*512 + d*64 + e  (head, depth, e ordered)
  wv    [128, 6, 384]          Wv slice transposed (ki, ko, (h e))
  qb/kb [128, 24]              permuted bias slices (per-partition, per chunk)
  out   [6, 2, 128, 4, 4, 64]  (h, dgrp, qp, qchunk, dsub, e), fp32

Pipeline per head: one DMA each for the head's 4 q and 4 k weight
chunks, project chunks into PSUM (6 accumulating matmuls each, DVE
copies to bf16 SBUF), then per depth-group: scoresT[ks,q] =
K'[e,ks].T @ Q'[e,q] into PSUM, the two depths of a pair issued
back-to-back on alternating PE row-groups 0-63/64-127 (same-row-group
back-to-back K=64 matmuls measured 1.7x slower); exp via ScalarE on
[128,1024] PSUM tiles (scale=1/8, per-partition mask bias) -> bf16
SBUF; AV with V augmented by a ones column so the same matmul yields
the softmax denominator in column 64; DVE reciprocal + broadcast-
multiply normalizes into query-chunk-pair tiles; one DMA out per pair.

All matmuls bf16 (fp32 runs at 1/4 rate on the PE; float32r and fp8
rejected: fp8 fails the 2e-2 accuracy budget, f32r measured slower).
Softmax needs no max-subtraction: scores are O(1) and the mask enters
as an additive bias, which cancels in the softmax ratio exactly like
the reference's where(mask==0, -1e9) + max-subtraction.
"""

import os
import sys

import numpy as np

try:
    import concourse.bass as bass  # noqa: F401
except ImportError:
    sys.path.insert(0, "/opt/trn_rl_repo")

import ml_dtypes

HIDDEN = 768
HEADS = 12
HD = 64
DEPTH = 8
B = 4
S = 512
N_CORES = 8
HB = HEADS // 2          # heads per core
P = 128
NKC = HIDDEN // P        # 6 contraction chunks
FC = HB * DEPTH * HD // P  # 24 feature chunks of Q'/K'
KSC = S // P             # 4 key/seq chunks
F = FC * P               # 3072

_BF = ml_dtypes.bfloat16

_cache = {}

DEFAULT_OPTS = dict(
    qk_bufs=2,        # SBUF bufs per q/k tag
    sc_bufs=2,        # PSUM score-tile bufs (2 banks each)
    av_in_pp=False,   # AV accumulates in the proj PSUM pool (measured bad)
    dgrp_loop=False,  # (head, depth-group)-granular proj->attention pipeline
    interleave=False,  # q,k,q,k projection order within a group
    copy_eng="vector",  # engine for Q/K PSUM->SBUF copies (GPSIMD can't read PSUM)
    ep_bufs=6,
    wts_bufs=2,       # per-head weight tile bufs (per q/k tag)
    ob_bufs=3,
    pp_bufs=2,
    batch_out=True,   # one output DMA per query-chunk pair
    consts_bufs=2,    # double-buffer hsT/wv/V across loop iterations
)


def _build(use_mask, use_qk_bias, reps=1, probe=None, tune=0, opts=None):
    import contextlib

    import concourse.bacc as bacc
    import concourse.mybir as mybir
    import concourse.tile as tile

    o = dict(DEFAULT_OPTS)
    if opts:
        o.update(opts)

    f32 = mybir.dt.float32
    bf = mybir.dt.bfloat16
    Exp = mybir.ActivationFunctionType.Exp

    nc = bacc.Bacc(
        "TRN2", target_bir_lowering=False, debug=False, num_devices=N_CORES
    )
    hsT_d = nc.dram_tensor("hsT", [P, NKC, S], bf, kind="ExternalInput")
    wq_d = nc.dram_tensor("wq", [HB, P, 4, NKC, P], bf, kind="ExternalInput")
    wk_d = nc.dram_tensor("wk", [HB, P, 4, NKC, P], bf, kind="ExternalInput")
    wv_d = nc.dram_tensor("wv", [P, NKC, HB * HD], bf, kind="ExternalInput")
    if use_qk_bias:
        qb_d = nc.dram_tensor("qb", [P, FC], f32, kind="ExternalInput")
        kb_d = nc.dram_tensor("kb", [P, FC], f32, kind="ExternalInput")
    if use_mask:
        mb_d = nc.dram_tensor("mb", [P, KSC], f32, kind="ExternalInput")
    out_d = nc.dram_tensor(
        "out", [HB, 2, P, KSC, 4, HD], f32, kind="ExternalOutput"
    )

    with tile.TileContext(nc) as tc:
        with (
            tc.tile_pool(name="consts", bufs=o["consts_bufs"]) as consts,
            tc.tile_pool(name="wts", bufs=o["wts_bufs"]) as wts,
            tc.tile_pool(name="qk", bufs=o["qk_bufs"]) as qk,
            tc.tile_pool(name="ep", bufs=o["ep_bufs"]) as ep,
            tc.tile_pool(name="ob", bufs=o["ob_bufs"]) as ob,
            tc.tile_pool(name="pp", bufs=o["pp_bufs"], space="PSUM") as ps_pp,
            tc.tile_pool(name="sc", bufs=o["sc_bufs"], space="PSUM") as ps_sc,
            (
                tc.tile_pool(name="av", bufs=2, space="PSUM")
                if not o["av_in_pp"]
                else contextlib.nullcontext()
            ) as _ps_av,
            tc.For_i(0, reps, 1) if reps > 1 else contextlib.nullcontext(),
        ):
            copy_eng = getattr(nc, o["copy_eng"])
            hsT_sb = consts.tile([P, NKC, S], bf, tag="hsT")
            nc.sync.dma_start(hsT_sb[:], hsT_d.ap())
            wv_sb = consts.tile([P, NKC, HB * HD], bf, tag="wv")
            nc.sync.dma_start(wv_sb[:], wv_d.ap())
            if use_qk_bias:
                qb_sb = consts.tile([P, FC], f32, tag="qb")
                nc.sync.dma_start(qb_sb[:], qb_d.ap())
                kb_sb = consts.tile([P, FC], f32, tag="kb")
                nc.sync.dma_start(kb_sb[:], kb_d.ap())
            if use_mask:
                mb_sb = consts.tile([P, KSC], f32, tag="mb")
                nc.sync.dma_start(mb_sb[:], mb_d.ap())

            v_sb = consts.tile([P, KSC, HB, HD + 1], bf, tag="v")
            nc.vector.memset(v_sb[:, :, :, HD : HD + 1], 1.0)

            # V projection first: psum[s128, (h e)384] = hsT_chunk.T @ wv
            for sc_ in range(KSC):
                ps = ps_pp.tile([P, S], f32, tag="pp")
                for ko in range(NKC):
                    nc.tensor.matmul(
                        ps[:, : HB * HD],
                        hsT_sb[:, ko, sc_ * P : (sc_ + 1) * P],
                        wv_sb[:, ko, :],
                        start=(ko == 0),
                        stop=(ko == NKC - 1),
                    )
                nc.vector.tensor_copy(
                    v_sb[:, sc_, :, 0:HD],
                    ps[:, : HB * HD].rearrange("p (h e) -> p h e", e=HD),
                )

            def project(wt, dst, b_name, h, lc, li):
                # wt[:, lc] holds chunk c = h*4+lc; write dst[:, li, :]
                ps = ps_pp.tile([P, S], f32, tag="pp")
                n_proj = 2 if probe == "proj2" else 1
                if probe == "projnone":
                    n_proj = 0
                    nc.vector.memset(ps[:, 0:4], 0.5)
                for _rep in range(n_proj):
                    for ko in range(NKC):
                        nc.tensor.matmul(
                            ps[:],
                            wt[:, lc, ko, :],
                            hsT_sb[:, ko, :],
                            start=(ko == 0),
                            stop=(ko == NKC - 1),
                        )
                if use_qk_bias:
                    c = h * 4 + lc
                    b_sb = qb_sb if b_name == "q" else kb_sb
                    nc.vector.tensor_scalar_add(
                        dst[:, li, :], ps[:], b_sb[:, c : c + 1]
                    )
                else:
                    copy_eng.tensor_copy(dst[:, li, :], ps[:])

            def scores_phase(h, dgrp, q_t, k_t, li_of):
                # li_of[pi] = index into q_t/k_t's chunk dim for depth-pair pi
                e_pairs = []
                for pi in range(2):
                    li = li_of[pi]
                    e_p = ep.tile([P, KSC, 2, S], bf, tag="exp")
                    for ksc in range(KSC):
                        sps = ps_sc.tile([P, 2, S], f32, tag="sc")
                        n_sc = 2 if probe == "sc2" else 1
                        if probe == "scnone":
                            n_sc = 0
                            nc.vector.memset(sps[:, :, 0:4], 0.5)
                        for _rep in range(n_sc):
                            for pd in range(2):
                                base = pd * 64
                                nc.tensor.matmul(
                                    sps[:, pd, :],
                                    k_t[
                                        base : base + 64,
                                        li,
                                        ksc * P : (ksc + 1) * P,
                                    ],
                                    q_t[base : base + 64, li, :],
                                    start=True,
                                    stop=True,
                                )
                        if probe == "exp2":
                            e_x = ep.tile([P, 2, S], bf, tag="exp2")
                            nc.scalar.activation(
                                e_x[:], sps[:], Exp, scale=0.125
                            )
                        if probe == "exphalf":
                            if ksc == 0:
                                nc.vector.memset(e_p[:, :, 1, 0:4], 0.5)
                            nc.scalar.activation(
                                e_p[:, ksc, 0, :],
                                sps[:, 0, :],
                                Exp,
                                scale=0.125,
                            )
                        elif use_mask:
                            nc.scalar.activation(
                                e_p[:, ksc, :, :],
                                sps[:],
                                Exp,
                                bias=mb_sb[:, ksc : ksc + 1],
                                scale=0.125,
                            )
                        else:
                            nc.scalar.activation(
                                e_p[:, ksc, :, :],
                                sps[:],
                                Exp,
                                scale=0.125,
                            )
                    e_pairs.append(e_p)
                return e_pairs

            def av_phase(h, dgrp, e_pairs):
                for qcp in range(2) if o["batch_out"] else range(4):
                    if o["batch_out"]:
                        o_t = ob.tile([P, 2, 4, HD], f32, tag="o")
                        qcs = (qcp * 2, qcp * 2 + 1)
                    else:
                        o_t = ob.tile([P, 1, 4, HD], f32, tag="o")
                        qcs = (qcp,)
                    for oi, qc in enumerate(qcs):
                        if o["av_in_pp"]:
                            avf = ps_pp.tile([P, S], f32, tag="pp")
                            av = avf[:, : 4 * (HD + 1)].rearrange(
                                "p (i e) -> p i e", e=HD + 1
                            )
                        else:
                            av = _ps_av.tile([P, 4, HD + 1], f32, tag="av")
                        for i in range(4):
                            pi, pd = i // 2, i % 2
                            for ksc in range(KSC):
                                nc.tensor.matmul(
                                    av[:, i, :],
                                    e_pairs[pi][
                                        :, ksc, pd, qc * P : (qc + 1) * P
                                    ],
                                    v_sb[:, ksc, h, :],
                                    start=(ksc == 0),
                                    stop=(ksc == KSC - 1),
                                )
                        r = ob.tile([P, 4], f32, tag="r")
                        nc.vector.reciprocal(r[:], av[:, :, HD])
                        nc.vector.tensor_mul(
                            o_t[:, oi],
                            av[:, :, 0:HD],
                            r[:, :, None].to_broadcast([P, 4, HD]),
                        )
                    out_eng = nc.scalar if o.get("dma_out_act") else nc.sync
                    out_eng.dma_start(
                        out_d.ap()[h, dgrp][
                            :, qcs[0] : qcs[-1] + 1
                        ],
                        o_t[:, : len(qcs)],
                    )

            def attention(h, dgrp, q_t, k_t, li_of):
                e_pairs = scores_phase(h, dgrp, q_t, k_t, li_of)
                if probe == "noav":
                    return
                av_phase(h, dgrp, e_pairs)

            if o.get("sw_pipe"):
                # Software-pipelined issue order: projection units are slotted
                # between scores(d) and AV(d) so the in-order PE queue has
                # filler work while the ScalarE exp tail completes; otherwise
                # AV blocks the queue head for the exp latency every dgrp.
                def alloc_head(h):
                    wt_q = wts.tile([P, 4, NKC, P], bf, tag="wq")
                    nc.sync.dma_start(wt_q[:], wq_d.ap()[h])
                    wt_k = wts.tile([P, 4, NKC, P], bf, tag="wk")
                    nc.sync.dma_start(wt_k[:], wk_d.ap()[h])
                    q_t = qk.tile([P, 4, S], bf, tag="q")
                    k_t = qk.tile([P, 4, S], bf, tag="k")
                    return wt_q, wt_k, q_t, k_t

                def proj_pair(h, lc, tl):
                    project(tl[0], tl[2], "q", h, lc, lc)
                    project(tl[1], tl[3], "k", h, lc, lc)

                tl = alloc_head(0)
                proj_pair(0, 0, tl)
                proj_pair(0, 1, tl)
                ntl = None
                for h in range(HB):
                    for d in range(2):
                        e = scores_phase(
                            h, d, tl[2], tl[3], li_of=[2 * d, 2 * d + 1]
                        )
                        if d == 0:
                            proj_pair(h, 2, tl)
                            proj_pair(h, 3, tl)
                        elif h + 1 < HB:
                            ntl = alloc_head(h + 1)
                            proj_pair(h + 1, 0, ntl)
                            proj_pair(h + 1, 1, ntl)
                        if probe != "noav":
                            av_phase(h, d, e)
                    if h + 1 < HB:
                        tl = ntl
            elif o["dgrp_loop"]:
                for h in range(HB):
                    for dgrp in range(2):
                        wt_q = wts.tile([P, 2, NKC, P], bf, tag="wq")
                        nc.sync.dma_start(
                            wt_q[:], wq_d.ap()[h][:, dgrp * 2 : dgrp * 2 + 2]
                        )
                        wt_k = wts.tile([P, 2, NKC, P], bf, tag="wk")
                        nc.sync.dma_start(
                            wt_k[:], wk_d.ap()[h][:, dgrp * 2 : dgrp * 2 + 2]
                        )
                        q_t = qk.tile([P, 2, S], bf, tag="q")
                        k_t = qk.tile([P, 2, S], bf, tag="k")
                        for pi in range(2):
                            lc = dgrp * 2 + pi
                            project(wt_q, q_t, "q", h, pi, pi)
                            project(wt_k, k_t, "k", h, pi, pi)
                        attention(h, dgrp, q_t, k_t, li_of=[0, 1])
            else:
                for h in range(HB):
                    wt_q = wts.tile([P, 4, NKC, P], bf, tag="wq")
                    nc.sync.dma_start(wt_q[:], wq_d.ap()[h])
                    wt_k = wts.tile([P, 4, NKC, P], bf, tag="wk")
                    nc.sync.dma_start(wt_k[:], wk_d.ap()[h])
                    q_t = qk.tile([P, 4, S], bf, tag="q")
                    k_t = qk.tile([P, 4, S], bf, tag="k")
                    if o["interleave"]:
                        for lc in range(4):
                            project(wt_q, q_t, "q", h, lc, lc)
                            project(wt_k, k_t, "k", h, lc, lc)
                    else:
                        for lc in range(4):
                            project(wt_q, q_t, "q", h, lc, lc)
                        for lc in range(4):
                            project(wt_k, k_t, "k", h, lc, lc)
                    for dgrp in range(2):
                        attention(
                            h, dgrp, q_t, k_t, li_of=[dgrp * 2, dgrp * 2 + 1]
                        )

    nc.compile()
    return nc


def _get_program(use_mask, use_qk_bias):
    key = (use_mask, use_qk_bias)
    if key not in _cache:
        _cache[key] = _build(use_mask, use_qk_bias)
    return _cache[key]


def _perm_idx(h0):
    # f' = h*512 + d*64 + e maps to original row ((h0+h)*64+e)*8 + d
    idx = np.empty(F, dtype=np.int64)
    f = 0
    for h in range(HB):
        for d in range(DEPTH):
            for e in range(HD):
                idx[f] = ((h0 + h) * HD + e) * DEPTH + d
                f += 1
    return idx


def _prep_w(Wd, idx):
    # [6144,768] -> permuted rows [3072,768] -> (h, ki, c, ko, f)
    A = np.ascontiguousarray(Wd[idx])  # [3072, 768]
    chunks = A.reshape(FC, P, NKC, P).transpose(0, 3, 2, 1)  # (fc, ki, ko, f)
    return np.ascontiguousarray(
        chunks.reshape(HB, 4, P, NKC, P).transpose(0, 2, 1, 3, 4)
    ).astype(_BF)


def _prep_hsT(hs_b):
    # [512, 768] -> [768,512] -> (ki, ko, s)
    return np.ascontiguousarray(
        hs_b.T.reshape(NKC, P, S).transpose(1, 0, 2)
    ).astype(_BF)


def _prep_wv(Wv, h0):
    Wvs = Wv[h0 * HD : (h0 + HB) * HD]  # [384, 768]
    return np.ascontiguousarray(
        Wvs.T.reshape(NKC, P, HB * HD).transpose(1, 0, 2)
    ).astype(_BF)


last_results = None


def kernel(
    hidden_states,
    attention_mask,
    Wq,
    bq,
    Wk,
    bk,
    Wv,
    bv,
    Wqd,
    bqd,
    Wkd,
    bkd,
):
    global last_results
    from concourse.bass_utils import run_bass_kernel_spmd

    hs = np.asarray(hidden_states, dtype=np.float32)
    mask = np.asarray(attention_mask)
    Wv = np.asarray(Wv, dtype=np.float32)
    bv = np.asarray(bv, dtype=np.float32)
    Wqd = np.asarray(Wqd, dtype=np.float32)
    bqd = np.asarray(bqd, dtype=np.float32)
    Wkd = np.asarray(Wkd, dtype=np.float32)
    bkd = np.asarray(bkd, dtype=np.float32)

    use_mask = not bool(np.all(mask != 0))
    use_qk_bias = bool(np.any(bqd) or np.any(bkd))
    nc = _get_program(use_mask, use_qk_bias)

    idx = [_perm_idx(0), _perm_idx(HB)]
    wq_p = [_prep_w(Wqd, idx[hh]) for hh in range(2)]
    wk_p = [_prep_w(Wkd, idx[hh]) for hh in range(2)]
    wv_p = [_prep_wv(Wv, hh * HB) for hh in range(2)]
    qb_p = [
        np.ascontiguousarray(bqd[idx[hh]].reshape(FC, P).T).astype(np.float32)
        for hh in range(2)
    ]
    kb_p = [
        np.ascontiguousarray(bkd[idx[hh]].reshape(FC, P).T).astype(np.float32)
        for hh in range(2)
    ]

    in_maps = []
    for c in range(N_CORES):
        b, hh = c // 2, c % 2
        m = {
            "hsT": _prep_hsT(hs[b]),
            "wq": wq_p[hh],
            "wk": wk_p[hh],
            "wv": wv_p[hh],
        }
        if use_qk_bias:
            m["qb"] = qb_p[hh]
            m["kb"] = kb_p[hh]
        if use_mask:
            mb = np.where(mask[b] == 0, np.float32(-1e9), np.float32(0.0))
            m["mb"] = np.ascontiguousarray(
                mb.reshape(KSC, P).T
            ).astype(np.float32)
        in_maps.append(m)

    res = run_bass_kernel_spmd(nc, in_maps, list(range(N_CORES)))
    last_results = res

    out = np.empty((DEPTH, B, HEADS, S, HD), dtype=np.float32)
    for c in range(N_CORES):
        b, hh = c // 2, c % 2
        arr = res.results[c]["out"]  # [6, 2, 128, 4, 4, 64] (h,dgrp,qp,qc,i,e)
        a = np.ascontiguousarray(arr.transpose(1, 4, 0, 3, 2, 5)).reshape(
            DEPTH, HB, S, HD
        )
        out[:, b, hh * HB : (hh + 1) * HB] = a
    if np.any(bv):
        out += bv.reshape(HEADS, HD)[None, None, :, None, :]
    return out


# revision 10
# speedup vs baseline: 1.0375x; 1.0131x over previous
"""Multi-depth attention (BaseMAWAttention) Trainium2 kernel.

Sharding: 8 cores = 4 batches x 2 head-halves (6 heads each). Each core
computes its (batch, head-half) slice end-to-end: QKV projections,
per-(head,depth) scores, softmax, AV. No collectives.

Device layouts (per core):
  hsT   [128, 6, 512]          hidden_states[b].T, k-major (ki, ko, s), bf16
  wq/wk [6, 128, 4, 6, 128]    permuted-transposed Wqd/Wkd: (h, ki, c, ko, f)
                               f' = h*512 + d*64 + e  (head, depth, e ordered)
  wv    [128, 6, 384]          Wv slice transposed (ki, ko, (h e))
  qb/kb [128, 24]              permuted bias slices (per-partition, per chunk)
  out   [6, 2, 128, 4, 4, 64]  (h, dgrp, qp, qchunk, dsub, e), fp32

Pipeline per head: one DMA each for the head's 4 q and 4 k weight
chunks, project chunks into PSUM (6 accumulating matmuls each, DVE
copies to bf16 SBUF), then per depth-group: scoresT[ks,q] =
K'[e,ks].T @ Q'[e,q] into PSUM, the two depths of a pair issued
back-to-back on alternating PE row-groups 0-63/64-127 (same-row-group
back-to-back K=64 matmuls measured 1.7x slower); exp via ScalarE on
[128,1024] PSUM tiles (scale=1/8, per-partition mask bias) -> bf16
SBUF; AV with V augmented by a ones column so the same matmul yields
the softmax denominator in column 64; DVE reciprocal + broadcast-
multiply normalizes into query-chunk-pair tiles; one DMA out per pair.

All matmuls bf16 (fp32 runs at 1/4 rate on the PE; float32r and fp8
rejected: fp8 fails the 2e-2 accuracy budget, f32r measured slower).
Softmax needs no max-subtraction: scores are O(1) and the mask enters
as an additive bias, which cancels in the softmax ratio exactly like
the reference's where(mask==0, -1e9) + max-subtraction.
"""

import os
import sys

import numpy as np

try:
    import concourse.bass as bass  # noqa: F401
except ImportError:
    sys.path.insert(0, "/opt/trn_rl_repo")

import ml_dtypes

HIDDEN = 768
HEADS = 12
HD = 64
DEPTH = 8
B = 4
S = 512
N_CORES = 8
HB = HEADS // 2          # heads per core
P = 128
NKC = HIDDEN // P        # 6 contraction chunks
FC = HB * DEPTH * HD // P  # 24 feature chunks of Q'/K'
KSC = S // P             # 4 key/seq chunks
F = FC * P               # 3072

_BF = ml_dtypes.bfloat16

_cache = {}

DEFAULT_OPTS = dict(
    qk_bufs=2,        # SBUF bufs per q/k tag
    sc_bufs=2,        # PSUM score-tile bufs (2 banks each)
    av_in_pp=False,   # AV accumulates in the proj PSUM pool (measured bad)
    dgrp_loop=False,  # (head, depth-group)-granular proj->attention pipeline
    interleave=False,  # q,k,q,k projection order within a group
    copy_eng="vector",  # engine for Q/K PSUM->SBUF copies (GPSIMD can't read PSUM)
    ep_bufs=6,
    wts_bufs=2,       # per-head weight tile bufs (per q/k tag)
    ob_bufs=3,
    pp_bufs=2,
    batch_out=True,   # one output DMA per query-chunk pair
    consts_bufs=2,    # double-buffer hsT/wv/V across loop iterations
)


def _build(use_mask, use_qk_bias, reps=1, probe=None, tune=0, opts=None):
    import contextlib

    import concourse.bacc as bacc
    import concourse.mybir as mybir
    import concourse.tile as tile

    o = dict(DEFAULT_OPTS)
    if opts:
        o.update(opts)

    f32 = mybir.dt.float32
    bf = mybir.dt.bfloat16
    Exp = mybir.ActivationFunctionType.Exp

    nc = bacc.Bacc(
        "TRN2", target_bir_lowering=False, debug=False, num_devices=N_CORES
    )
    hsT_d = nc.dram_tensor("hsT", [P, NKC, S], bf, kind="ExternalInput")
    wq_d = nc.dram_tensor("wq", [HB, P, 4, NKC, P], bf, kind="ExternalInput")
    wk_d = nc.dram_tensor("wk", [HB, P, 4, NKC, P], bf, kind="ExternalInput")
    wv_d = nc.dram_tensor("wv", [P, NKC, HB * HD], bf, kind="ExternalInput")
    if use_qk_bias:
        qb_d = nc.dram_tensor("qb", [P, FC], f32, kind="ExternalInput")
        kb_d = nc.dram_tensor("kb", [P, FC], f32, kind="ExternalInput")
    if use_mask:
        mb_d = nc.dram_tensor("mb", [P, KSC], f32, kind="ExternalInput")
    out_d = nc.dram_tensor(
        "out", [HB, 2, P, KSC, 4, HD], f32, kind="ExternalOutput"
    )

    with tile.TileContext(nc) as tc:
        with (
            tc.tile_pool(name="consts", bufs=o["consts_bufs"]) as consts,
            tc.tile_pool(name="wts", bufs=o["wts_bufs"]) as wts,
            tc.tile_pool(name="qk", bufs=o["qk_bufs"]) as qk,
            tc.tile_pool(name="ep", bufs=o["ep_bufs"]) as ep,
            tc.tile_pool(name="ob", bufs=o["ob_bufs"]) as ob,
            tc.tile_pool(name="pp", bufs=o["pp_bufs"], space="PSUM") as ps_pp,
            tc.tile_pool(name="sc", bufs=o["sc_bufs"], space="PSUM") as ps_sc,
            (
                tc.tile_pool(name="av", bufs=2, space="PSUM")
                if not o["av_in_pp"]
                else contextlib.nullcontext()
            ) as _ps_av,
            tc.For_i(0, reps, 1) if reps > 1 else contextlib.nullcontext(),
        ):
            copy_eng = getattr(nc, o["copy_eng"])
            hsT_sb = consts.tile([P, NKC, S], bf, tag="hsT")
            nc.sync.dma_start(hsT_sb[:], hsT_d.ap())
            wv_sb = consts.tile([P, NKC, HB * HD], bf, tag="wv")
            nc.sync.dma_start(wv_sb[:], wv_d.ap())
            if use_qk_bias:
                qb_sb = consts.tile([P, FC], f32, tag="qb")
                nc.sync.dma_start(qb_sb[:], qb_d.ap())
                kb_sb = consts.tile([P, FC], f32, tag="kb")
                nc.sync.dma_start(kb_sb[:], kb_d.ap())
            if use_mask:
                mb_sb = consts.tile([P, KSC], f32, tag="mb")
                nc.sync.dma_start(mb_sb[:], mb_d.ap())

            v_sb = consts.tile([P, KSC, HB, HD + 1], bf, tag="v")
            nc.vector.memset(v_sb[:, :, :, HD : HD + 1], 1.0)

            # V projection first: psum[s128, (h e)384] = hsT_chunk.T @ wv
            for sc_ in range(KSC):
                ps = ps_pp.tile([P, S], f32, tag="pp")
                for ko in range(NKC):
                    nc.tensor.matmul(
                        ps[:, : HB * HD],
                        hsT_sb[:, ko, sc_ * P : (sc_ + 1) * P],
                        wv_sb[:, ko, :],
                        start=(ko == 0),
                        stop=(ko == NKC - 1),
                    )
                nc.vector.tensor_copy(
                    v_sb[:, sc_, :, 0:HD],
                    ps[:, : HB * HD].rearrange("p (h e) -> p h e", e=HD),
                )

            def project(wt, dst, b_name, h, lc, li):
                # wt[:, lc] holds chunk c = h*4+lc; write dst[:, li, :]
                ps = ps_pp.tile([P, S], f32, tag="pp")
                n_proj = 2 if probe == "proj2" else 1
                if probe == "projnone":
                    n_proj = 0
                    nc.vector.memset(ps[:, 0:4], 0.5)
                for _rep in range(n_proj):
                    for ko in range(NKC):
                        nc.tensor.matmul(
                            ps[:],
                            wt[:, lc, ko, :],
                            hsT_sb[:, ko, :],
                            start=(ko == 0),
                            stop=(ko == NKC - 1),
                        )
                if use_qk_bias:
                    c = h * 4 + lc
                    b_sb = qb_sb if b_name == "q" else kb_sb
                    nc.vector.tensor_scalar_add(
                        dst[:, li, :], ps[:], b_sb[:, c : c + 1]
                    )
                else:
                    copy_eng.tensor_copy(dst[:, li, :], ps[:])

            def scores_phase(h, dgrp, q_t, k_t, li_of):
                # li_of[pi] = index into q_t/k_t's chunk dim for depth-pair pi
                e_pairs = []
                for pi in range(2):
                    li = li_of[pi]
                    e_p = ep.tile([P, KSC, 2, S], bf, tag="exp")
                    for ksc in range(KSC):
                        sps = ps_sc.tile([P, 2, S], f32, tag="sc")
                        n_sc = 2 if probe == "sc2" else 1
                        if probe == "scnone":
                            n_sc = 0
                            nc.vector.memset(sps[:, :, 0:4], 0.5)
                        for _rep in range(n_sc):
                            for pd in range(2):
                                base = pd * 64
                                nc.tensor.matmul(
                                    sps[:, pd, :],
                                    k_t[
                                        base : base + 64,
                                        li,
                                        ksc * P : (ksc + 1) * P,
                                    ],
                                    q_t[base : base + 64, li, :],
                                    start=True,
                                    stop=True,
                                )
                        if probe == "exp2":
                            e_x = ep.tile([P, 2, S], bf, tag="exp2")
                            nc.scalar.activation(
                                e_x[:], sps[:], Exp, scale=0.125
                            )
                        if probe == "exphalf":
                            if ksc == 0:
                                nc.vector.memset(e_p[:, :, 1, 0:4], 0.5)
                            nc.scalar.activation(
                                e_p[:, ksc, 0, :],
                                sps[:, 0, :],
                                Exp,
                                scale=0.125,
                            )
                        elif use_mask:
                            nc.scalar.activation(
                                e_p[:, ksc, :, :],
                                sps[:],
                                Exp,
                                bias=mb_sb[:, ksc : ksc + 1],
                                scale=0.125,
                            )
                        else:
                            nc.scalar.activation(
                                e_p[:, ksc, :, :],
                                sps[:],
                                Exp,
                                scale=0.125,
                            )
                    e_pairs.append(e_p)
                return e_pairs

            def av_phase(h, dgrp, e_pairs):
                for qcp in range(2) if o["batch_out"] else range(4):
                    if o["batch_out"]:
                        o_t = ob.tile([P, 2, 4, HD], f32, tag="o")
                        qcs = (qcp * 2, qcp * 2 + 1)
                    else:
                        o_t = ob.tile([P, 1, 4, HD], f32, tag="o")
                        qcs = (qcp,)
                    if o.get("avx") and len(qcs) == 2 and not o["av_in_pp"]:
                        # interleave the two query chunks' accumulation
                        # chains so consecutive chains alternate PSUM banks
                        avs = []
                        for oi in range(2):
                            av_t = _ps_av.tile([P, 4, HD + 1], f32, tag="av")
                            avs.append(av_t)
                        for i in range(4):
                            pi, pd = i // 2, i % 2
                            for oi, qc in enumerate(qcs):
                                for ksc in range(KSC):
                                    nc.tensor.matmul(
                                        avs[oi][:, i, :],
                                        e_pairs[pi][
                                            :, ksc, pd, qc * P : (qc + 1) * P
                                        ],
                                        v_sb[:, ksc, h, :],
                                        start=(ksc == 0),
                                        stop=(ksc == KSC - 1),
                                    )
                        for oi in range(2):
                            r = ob.tile([P, 4], f32, tag="r")
                            nc.vector.reciprocal(r[:], avs[oi][:, :, HD])
                            nc.vector.tensor_mul(
                                o_t[:, oi],
                                avs[oi][:, :, 0:HD],
                                r[:, :, None].to_broadcast([P, 4, HD]),
                            )
                        nc.sync.dma_start(
                            out_d.ap()[h, dgrp][:, qcs[0] : qcs[-1] + 1],
                            o_t[:, :2],
                        )
                        continue
                    for oi, qc in enumerate(qcs):
                        if o["av_in_pp"]:
                            avf = ps_pp.tile([P, S], f32, tag="pp")
                            av = avf[:, : 4 * (HD + 1)].rearrange(
                                "p (i e) -> p i e", e=HD + 1
                            )
                        else:
                            av = _ps_av.tile([P, 4, HD + 1], f32, tag="av")
                        for i in range(4):
                            pi, pd = i // 2, i % 2
                            for ksc in range(KSC):
                                nc.tensor.matmul(
                                    av[:, i, :],
                                    e_pairs[pi][
                                        :, ksc, pd, qc * P : (qc + 1) * P
                                    ],
                                    v_sb[:, ksc, h, :],
                                    start=(ksc == 0),
                                    stop=(ksc == KSC - 1),
                                )
                        r = ob.tile([P, 4], f32, tag="r")
                        nc.vector.reciprocal(r[:], av[:, :, HD])
                        nc.vector.tensor_mul(
                            o_t[:, oi],
                            av[:, :, 0:HD],
                            r[:, :, None].to_broadcast([P, 4, HD]),
                        )
                    out_eng = nc.scalar if o.get("dma_out_act") else nc.sync
                    out_eng.dma_start(
                        out_d.ap()[h, dgrp][
                            :, qcs[0] : qcs[-1] + 1
                        ],
                        o_t[:, : len(qcs)],
                    )

            def attention(h, dgrp, q_t, k_t, li_of):
                e_pairs = scores_phase(h, dgrp, q_t, k_t, li_of)
                if probe == "noav":
                    return
                av_phase(h, dgrp, e_pairs)

            if o.get("sw_pipe"):
                # Software-pipelined issue order: projection units are slotted
                # between scores(d) and AV(d) so the in-order PE queue has
                # filler work while the ScalarE exp tail completes; otherwise
                # AV blocks the queue head for the exp latency every dgrp.
                def alloc_head(h):
                    wt_q = wts.tile([P, 4, NKC, P], bf, tag="wq")
                    nc.sync.dma_start(wt_q[:], wq_d.ap()[h])
                    wt_k = wts.tile([P, 4, NKC, P], bf, tag="wk")
                    nc.sync.dma_start(wt_k[:], wk_d.ap()[h])
                    q_t = qk.tile([P, 4, S], bf, tag="q")
                    k_t = qk.tile([P, 4, S], bf, tag="k")
                    return wt_q, wt_k, q_t, k_t

                def proj_pair(h, lc, tl):
                    project(tl[0], tl[2], "q", h, lc, lc)
                    project(tl[1], tl[3], "k", h, lc, lc)

                tl = alloc_head(0)
                proj_pair(0, 0, tl)
                proj_pair(0, 1, tl)
                ntl = None
                for h in range(HB):
                    for d in range(2):
                        e = scores_phase(
                            h, d, tl[2], tl[3], li_of=[2 * d, 2 * d + 1]
                        )
                        if d == 0:
                            proj_pair(h, 2, tl)
                            proj_pair(h, 3, tl)
                        elif h + 1 < HB:
                            ntl = alloc_head(h + 1)
                            proj_pair(h + 1, 0, ntl)
                            proj_pair(h + 1, 1, ntl)
                        if probe != "noav":
                            av_phase(h, d, e)
                    if h + 1 < HB:
                        tl = ntl
            elif o["dgrp_loop"]:
                for h in range(HB):
                    for dgrp in range(2):
                        wt_q = wts.tile([P, 2, NKC, P], bf, tag="wq")
                        nc.sync.dma_start(
                            wt_q[:], wq_d.ap()[h][:, dgrp * 2 : dgrp * 2 + 2]
                        )
                        wt_k = wts.tile([P, 2, NKC, P], bf, tag="wk")
                        nc.sync.dma_start(
                            wt_k[:], wk_d.ap()[h][:, dgrp * 2 : dgrp * 2 + 2]
                        )
                        q_t = qk.tile([P, 2, S], bf, tag="q")
                        k_t = qk.tile([P, 2, S], bf, tag="k")
                        for pi in range(2):
                            lc = dgrp * 2 + pi
                            project(wt_q, q_t, "q", h, pi, pi)
                            project(wt_k, k_t, "k", h, pi, pi)
                        attention(h, dgrp, q_t, k_t, li_of=[0, 1])
            else:
                for h in range(HB):
                    wt_q = wts.tile([P, 4, NKC, P], bf, tag="wq")
                    nc.sync.dma_start(wt_q[:], wq_d.ap()[h])
                    wt_k = wts.tile([P, 4, NKC, P], bf, tag="wk")
                    nc.sync.dma_start(wt_k[:], wk_d.ap()[h])
                    q_t = qk.tile([P, 4, S], bf, tag="q")
                    k_t = qk.tile([P, 4, S], bf, tag="k")
                    if o["interleave"]:
                        for lc in range(4):
                            project(wt_q, q_t, "q", h, lc, lc)
                            project(wt_k, k_t, "k", h, lc, lc)
                    else:
                        for lc in range(4):
                            project(wt_q, q_t, "q", h, lc, lc)
                        for lc in range(4):
                            project(wt_k, k_t, "k", h, lc, lc)
                    for dgrp in range(2):
                        attention(
                            h, dgrp, q_t, k_t, li_of=[dgrp * 2, dgrp * 2 + 1]
                        )

    nc.compile()
    return nc


def _get_program(use_mask, use_qk_bias):
    key = (use_mask, use_qk_bias)
    if key not in _cache:
        _cache[key] = _build(use_mask, use_qk_bias)
    return _cache[key]


def _perm_idx(h0):
    # f' = h*512 + d*64 + e maps to original row ((h0+h)*64+e)*8 + d
    idx = np.empty(F, dtype=np.int64)
    f = 0
    for h in range(HB):
        for d in range(DEPTH):
            for e in range(HD):
                idx[f] = ((h0 + h) * HD + e) * DEPTH + d
                f += 1
    return idx


def _prep_w(Wd, idx):
    # [6144,768] -> permuted rows [3072,768] -> (h, ki, c, ko, f)
    A = np.ascontiguousarray(Wd[idx])  # [3072, 768]
    chunks = A.reshape(FC, P, NKC, P).transpose(0, 3, 2, 1)  # (fc, ki, ko, f)
    return np.ascontiguousarray(
        chunks.reshape(HB, 4, P, NKC, P).transpose(0, 2, 1, 3, 4)
    ).astype(_BF)


def _prep_hsT(hs_b):
    # [512, 768] -> [768,512] -> (ki, ko, s)
    return np.ascontiguousarray(
        hs_b.T.reshape(NKC, P, S).transpose(1, 0, 2)
    ).astype(_BF)


def _prep_wv(Wv, h0):
    Wvs = Wv[h0 * HD : (h0 + HB) * HD]  # [384, 768]
    return np.ascontiguousarray(
        Wvs.T.reshape(NKC, P, HB * HD).transpose(1, 0, 2)
    ).astype(_BF)


last_results = None


def kernel(
    hidden_states,
    attention_mask,
    Wq,
    bq,
    Wk,
    bk,
    Wv,
    bv,
    Wqd,
    bqd,
    Wkd,
    bkd,
):
    global last_results
    from concourse.bass_utils import run_bass_kernel_spmd

    hs = np.asarray(hidden_states, dtype=np.float32)
    mask = np.asarray(attention_mask)
    Wv = np.asarray(Wv, dtype=np.float32)
    bv = np.asarray(bv, dtype=np.float32)
    Wqd = np.asarray(Wqd, dtype=np.float32)
    bqd = np.asarray(bqd, dtype=np.float32)
    Wkd = np.asarray(Wkd, dtype=np.float32)
    bkd = np.asarray(bkd, dtype=np.float32)

    use_mask = not bool(np.all(mask != 0))
    use_qk_bias = bool(np.any(bqd) or np.any(bkd))
    nc = _get_program(use_mask, use_qk_bias)

    idx = [_perm_idx(0), _perm_idx(HB)]
    wq_p = [_prep_w(Wqd, idx[hh]) for hh in range(2)]
    wk_p = [_prep_w(Wkd, idx[hh]) for hh in range(2)]
    wv_p = [_prep_wv(Wv, hh * HB) for hh in range(2)]
    qb_p = [
        np.ascontiguousarray(bqd[idx[hh]].reshape(FC, P).T).astype(np.float32)
        for hh in range(2)
    ]
    kb_p = [
        np.ascontiguousarray(bkd[idx[hh]].reshape(FC, P).T).astype(np.float32)
        for hh in range(2)
    ]

    in_maps = []
    for c in range(N_CORES):
        b, hh = c // 2, c % 2
        m = {
            "hsT": _prep_hsT(hs[b]),
            "wq": wq_p[hh],
            "wk": wk_p[hh],
            "wv": wv_p[hh],
        }
        if use_qk_bias:
            m["qb"] = qb_p[hh]
            m["kb"] = kb_p[hh]
        if use_mask:
            mb = np.where(mask[b] == 0, np.float32(-1e9), np.float32(0.0))
            m["mb"] = np.ascontiguousarray(
                mb.reshape(KSC, P).T
            ).astype(np.float32)
        in_maps.append(m)

    res = run_bass_kernel_spmd(nc, in_maps, list(range(N_CORES)))
    last_results = res

    out = np.empty((DEPTH, B, HEADS, S, HD), dtype=np.float32)
    for c in range(N_CORES):
        b, hh = c // 2, c % 2
        arr = res.results[c]["out"]  # [6, 2, 128, 4, 4, 64] (h,dgrp,qp,qc,i,e)
        a = np.ascontiguousarray(arr.transpose(1, 4, 0, 3, 2, 5)).reshape(
            DEPTH, HB, S, HD
        )
        out[:, b, hh * HB : (hh + 1) * HB] = a
    if np.any(bv):
        out += bv.reshape(HEADS, HD)[None, None, :, None, :]
    return out
